# revision 37
# baseline (speedup 1.0000x reference)
"""GCNBlock (GraphSAGE mean conv + LayerNorm) Trainium2 kernel, v2.

Problem shapes (hardcoded): B=8, N=8192, F_IN=F_OUT=64, 8 NeuronCores.

Math (reference):
    A    = (adj > 0)                      # [N, N], values in {0, 1}
    deg  = A.sum(1)
    agg  = (A @ x[b]) / max(deg, 1)       # per batch b
    out  = relu(x @ W_self + agg @ W_neigh (+ biases))
    out  = LayerNorm(out) * gamma + beta  # over feature dim, eps=1e-5

Sharding: 1D row partition; core c owns node rows [c*1024, (c+1)*1024).

v2 design (vs v1's 163 us):
  * The big A@x aggregation runs in fp8 DoubleRow perf mode (2 fp8 MACs per
    PE cell per cycle): both the adjacency and x are fp8e4m3.  x-quantization
    error is benign because the neigh path is ~64x smaller in magnitude than
    the self path (its W output scale is 1/sqrt(deg) vs 1).  Measured DR MM
    cadence: 216 ns for a [128x(2x128)] x [128x(2x512)] matmul = 1 virtual
    column/cycle = fp8 peak; the 55 us agg phase is the compute floor.
  * Operands are SWAPPED vs v1: xr (x in [j, bf] layout) is the stationary,
    the adjacency streams as the moving operand.  The product then lands
    already transposed (aggT[bf, i]) which kills all 64 PE transposes of v1,
    and each stationary serves 2 matmuls so LDWEIGHTS (256-col DR load, no
    FWL) hides completely under the MM stream.
  * deg: at-tiles are pair-summed on the (otherwise idle) DVE into ft[j,i]
    per 512-node half, then one ones-matmul reduces the partition dim.
  * LayerNorm runs in the transposed domain: mean and centered variance are
    feature-dim reductions = tiny [128,2] block-diagonal matmuls on the PE;
    per-node (free-dim) broadcasts of mu/rstd are [2,128] selector matmuls
    into PSUM, not DVE work.  Per-piece DVE is only 3 elementwise ops.
  * Output is written transposed ([bf, i]) and unshuffled on the host.

Schedule: agg phase (256 MMs back-to-back, folds riding on DVE, at/xr
streaming on separate rings) -> deg reduce + 1/max(deg,1) -> 8 backend
pieces (chunk x node-half) pipelined across PE/ACT/DVE.

gamma/beta are applied on the host (exact affine; ones/zeros here).

HW exec time: see test.py; target ~80 us (PE ~66 us busy).
"""

import numpy as np
import ml_dtypes

import concourse.bass as bass
import concourse.mybir as mybir
from concourse.tile import TileContext
from concourse.bass_utils import run_bass_kernel_spmd

B, N, F = 8, 8192, 64
N_CORES = 8
R = N // N_CORES          # rows (nodes) per core = 1024
JT = N // 128             # contraction tiles = 64
JP = JT // 2              # DoubleRow contraction pairs = 32
BF = B * F                # stacked batch*feature dim = 512
CH = BF // 128            # 128-wide chunks of the bf dim = 4
NH = 2                    # 512-node halves of the core's rows
LN_EPS = 1e-5

_F16 = mybir.dt.float16
_F32 = mybir.dt.float32
_F8 = mybir.dt.float8e4
_DR = mybir.MatmulPerfMode.DoubleRow


def _build_bass() -> bass.Bass:
    nc = bass.Bass()

    # Host-side layouts (see _prep_inputs):
    #   xr : [128 p, JT, BF]      fp8, xr[p, jt, b*64+f] = x[b, jt*128+p, f]
    #   ath: [NH, 128 p, JT, 512] fp8, ath[h, p, jt, i] = A[c*1024+h*512+i, jt*128+p]
    #   xt2: [CH, 128 p, R]       fp16, xt2[ch, p, i] = x^T in chunk layout
    xr = nc.dram_tensor("xr", (128, JT, BF), _F8, kind="ExternalInput")
    ath = nc.dram_tensor("ath", (NH, 128, JT, 512), _F8, kind="ExternalInput")
    xt2 = nc.dram_tensor("xt2", (CH, 128, R), _F16, kind="ExternalInput")
    wnblk = nc.dram_tensor("wnblk", (128, 128), _F16, kind="ExternalInput")
    wsblk = nc.dram_tensor("wsblk", (128, 128), _F16, kind="ExternalInput")
    bvec = nc.dram_tensor("bvec", (128, 1), _F32, kind="ExternalInput")
    blkc = nc.dram_tensor("blkc", (128, 2), _F16, kind="ExternalInput")
    selc = nc.dram_tensor("selc", (2, 128), _F16, kind="ExternalInput")
    epsc = nc.dram_tensor("epsc", (2, 1), _F32, kind="ExternalInput")
    outT = nc.dram_tensor("outT", (CH, 128, R), _F32, kind="ExternalOutput")

    SLICES = [2, 2, 4, 8, 8, 8, 8, 8, 8, 8]   # jt per DMA piece (all even)
    FOLD_PIECES = 5                            # at pieces folded on the DVE
    FOLD_JT = sum(SLICES[:FOLD_PIECES])        # = 24 jt (12 jp)

    from contextlib import ExitStack

    with TileContext(nc) as tc:
        with ExitStack() as es:
            consts = es.enter_context(tc.tile_pool(name="consts", bufs=1))
            xrp = es.enter_context(tc.tile_pool(name="xrp", bufs=len(SLICES)))
            atp = es.enter_context(tc.tile_pool(name="atp", bufs=20))
            xtp = es.enter_context(tc.tile_pool(name="xtp", bufs=CH))
            ftp = es.enter_context(tc.tile_pool(name="ftp", bufs=NH))
            php = es.enter_context(tc.tile_pool(name="php", bufs=5))
            qrp = es.enter_context(tc.tile_pool(name="qrp", bufs=1))
            sbp = es.enter_context(tc.tile_pool(name="sbp", bufs=2))
            aggrp = es.enter_context(tc.tile_pool(name="aggrp", bufs=8))
            aggsp = es.enter_context(tc.tile_pool(name="aggsp", bufs=3))
            rp = es.enter_context(tc.tile_pool(name="rp", bufs=4))
            dp = es.enter_context(tc.tile_pool(name="dp", bufs=6))
            smalls = es.enter_context(tc.tile_pool(name="smalls", bufs=3))
            pk1 = es.enter_context(tc.tile_pool(name="pk1", bufs=1))
            pk2 = es.enter_context(tc.tile_pool(name="pk2", bufs=2))
            outp = es.enter_context(tc.tile_pool(name="outp", bufs=3))
            # ---- constants -------------------------------------------------
            ones2 = consts.tile([128, 2], _F16)
            nc.vector.memset(ones2, 1.0)
            # fp8 all-ones stationary for DoubleRow degree matmuls; sliced
            # [:, :, 0:2] (free strides must be 16B-aligned, hence width 16).
            ones2dr = consts.tile([128, 2, 16], _F8)
            nc.vector.memset(ones2dr, 1.0)
            blk = consts.tile([128, 2], _F16)      # block mean weights (1/64)
            nc.gpsimd.dma_start(out=blk, in_=blkc[:, :])
            sel = consts.tile([2, 128], _F16)      # mu/rstd partition-bcast
            nc.gpsimd.dma_start(out=sel, in_=selc[:, :])
            eps2 = consts.tile([2, 1], _F32)
            nc.gpsimd.dma_start(out=eps2, in_=epsc[:, :])

            # ---- DMA kickoff (rings: ath on SP, xr on ACT, rest on DVE) ----
            xr_lut = []
            off = 0
            for k, sz in enumerate(SLICES):
                t = xrp.tile([128, sz, BF], _F8, name=f"xr{k}", tag="xr",
                             padded_shape=[128, 8, BF])
                nc.scalar.dma_start(out=t, in_=xr[:, off:off + sz, :])
                xr_lut.extend((t, l) for l in range(sz))
                off += sz
            at_lut = {h: [] for h in range(NH)}
            at_pieces = {h: [] for h in range(NH)}
            off = 0
            for k, sz in enumerate(SLICES):
                for h in range(NH):
                    t = atp.tile([128, sz, 512], _F8, name=f"at{h}_{k}",
                                 tag="at", padded_shape=[128, 8, 512])
                    nc.sync.dma_start(out=t, in_=ath[h, :, off:off + sz, :])
                    at_lut[h].extend((t, l) for l in range(sz))
                    at_pieces[h].append((t, sz))
                off += sz
            wn_sb = consts.tile([128, 128], _F16)
            nc.gpsimd.dma_start(out=wn_sb, in_=wnblk[:, :])
            ws_sb = consts.tile([128, 128], _F16)
            nc.gpsimd.dma_start(out=ws_sb, in_=wsblk[:, :])
            bias_sb = consts.tile([128, 1], _F32)
            nc.gpsimd.dma_start(out=bias_sb, in_=bvec[:, :])
            xt_sb = []
            for ch in range(CH):
                t = xtp.tile([128, R], _F16, name=f"xt{ch}", tag="xt")
                nc.gpsimd.dma_start(out=t, in_=xt2[ch])
                xt_sb.append(t)

            ft = [ftp.tile([128, 512], _F16, name=f"ft{h}", tag="ft")
                  for h in range(NH)]

            # ---- agg phase: 256 DoubleRow MMs + DVE degree folds -----------
            with tc.tile_pool(name="ps_agg", bufs=8, space="PSUM") as ps_agg:
                aggps = {(ch, h): ps_agg.tile([128, BF], _F32,
                                              name=f"agg{ch}{h}", tag="agg")
                         for ch in range(CH) for h in range(NH)}
                for q in range(JP):
                    xt_t, xl = xr_lut[2 * q]
                    xt_t2, xl2 = xr_lut[2 * q + 1]
                    assert xt_t2 is xt_t and xl2 == xl + 1
                    for ch in range(CH):
                        lhsT = xt_t[:, xl:xl + 2, ch * 128:(ch + 1) * 128]
                        for h in range(NH):
                            at_t, al = at_lut[h][2 * q]
                            at_t2, al2 = at_lut[h][2 * q + 1]
                            assert at_t2 is at_t and al2 == al + 1
                            nc.tensor.matmul(
                                aggps[(ch, h)], lhsT=lhsT,
                                rhs=at_t[:, al:al + 2, :],
                                start=(q == 0), stop=(q == JP - 1),
                                perf_mode=_DR,
                            )
                # Degree partial fold on the DVE for DMA pieces 0..4 (24 jt
                # per half), as a batched pair-sum tree (fp8 reads run at the
                # DVE's 1x tier, so batch FD large and keep levels shallow;
                # fp16 intermediates, counts <= 24, exact).  Pieces 5..9 are
                # handled by PE ones-matmuls in the backend scope.
                for h in range(NH):
                    phs = []
                    for k in range(FOLD_PIECES):
                        t, sz = at_pieces[h][k]
                        hf = sz // 2
                        ph = php.tile([128, hf, 512], _F8, tag="ph",
                                      padded_shape=[128, 4, 512])
                        nc.vector.tensor_add(out=ph, in0=t[:, 0:hf, :],
                                             in1=t[:, hf:sz, :])
                        phs.append((ph, hf))
                    # sizes now [1, 1, 2, 4, 4]; combine into ft[h] [128,512].
                    q1 = qrp.tile([128, 4, 512], _F16, tag="q")
                    nc.vector.tensor_add(out=q1, in0=phs[3][0], in1=phs[4][0])
                    r1 = qrp.tile([128, 2, 512], _F16, tag="r")
                    nc.vector.tensor_add(out=r1, in0=q1[:, 0:2, :],
                                         in1=q1[:, 2:4, :])
                    nc.vector.tensor_add(out=r1, in0=r1, in1=phs[2][0])
                    nc.vector.tensor_add(out=ft[h], in0=phs[0][0][:, 0, :],
                                         in1=phs[1][0][:, 0, :])
                    nc.vector.tensor_add(out=ft[h], in0=ft[h],
                                         in1=r1[:, 0, :])
                    nc.vector.tensor_add(out=ft[h], in0=ft[h],
                                         in1=r1[:, 1, :])

                # drain aggT to SBUF fp16 (raw; 1/deg applied later) to free
                # the PSUM banks for the backend pools.
                aggR = {}
                for ch in range(CH):
                    for h in range(NH):
                        t = aggrp.tile([128, BF], _F16, name=f"aR{ch}{h}",
                                       tag="aggR")
                        nc.scalar.copy(out=t, in_=aggps[(ch, h)])
                        aggR[(ch, h)] = t

            # ---- deg -> s = 1/max(deg,1), then backend pieces --------------
            with ExitStack() as es2:
                ps_tot = es2.enter_context(tc.tile_pool(name="ps_tot", bufs=2, space="PSUM"))
                ps_sm = es2.enter_context(tc.tile_pool(name="ps_sm", bufs=6, space="PSUM"))
                # Degree -> s = 1/max(deg, 1).  The DVE's iterative
                # reciprocal costs ~3.3us per call regardless of how few
                # partitions carry data, so both halves' clamped degrees are
                # packed into one [128, 512] tile at partition offsets 0/32
                # and inverted with a single call.
                dpk = pk1.tile([128, 512], _F32, tag="dpk")
                nc.vector.memset(dpk, 1.0)
                for h in range(NH):
                    # deg = folded pieces (via ones2 @ ft) + DoubleRow
                    # ones-matmuls over the unfolded at pieces 5..9.
                    degp = ps_sm.tile([2, 512], _F32, tag="pss", padded_shape=[128, 512])
                    nc.tensor.matmul(degp, lhsT=ones2, rhs=ft[h],
                                     start=True, stop=False,
                                     skip_group_check=True)
                    for jp in range(FOLD_JT // 2, JP):
                        at_t, al = at_lut[h][2 * jp]
                        at_t2, al2 = at_lut[h][2 * jp + 1]
                        assert at_t2 is at_t and al2 == al + 1
                        nc.tensor.matmul(
                            degp, lhsT=ones2dr[:, :, 0:2],
                            rhs=at_t[:, al:al + 2, :],
                            start=False, stop=(jp == JP - 1),
                            perf_mode=_DR, skip_group_check=True)
                    nc.vector.tensor_scalar_max(
                        out=dpk[32 * h:32 * h + 2, :], in0=degp, scalar1=1.0)
                spk = pk1.tile([128, 512], _F32, tag="spk")
                nc.vector.reciprocal(out=spk, in_=dpk)
                s_b = []
                for h in range(NH):
                    s2h = smalls.tile([2, 512], _F16, tag="s2h")
                    nc.scalar.copy(out=s2h, in_=spk[32 * h:32 * h + 2, :])
                    sbb = ps_sm.tile([128, 512], _F32, name=f"s_bp{h}",
                                     tag="pss")
                    nc.tensor.matmul(sbb, lhsT=sel, rhs=s2h,
                                     start=True, stop=True)
                    sbs = sbp.tile([128, 512], _F16, name=f"s_b{h}",
                                   tag="s_b")
                    nc.scalar.copy(out=sbs, in_=sbb)
                    s_b.append(sbs)

                # Backend pieces.  rstd reciprocals are likewise batched 4
                # pieces at a time (partition offsets 0/32/64/96).
                pieces = [(ch, h) for ch in range(CH) for h in range(NH)]
                front = {}
                vpk = None
                for k, (ch, h) in enumerate(pieces):
                    kk, grp = k % 4, k // 4
                    if kk == 0:
                        vpk = pk2.tile([128, 512], _F32, tag="vpk")
                        nc.vector.memset(vpk, 1.0)
                    aggS = aggsp.tile([128, BF], _F16, tag="aggS")
                    nc.vector.tensor_mul(out=aggS, in0=aggR[(ch, h)],
                                         in1=s_b[h])
                    tot = ps_tot.tile([128, 512], _F32, tag="tot")
                    nc.tensor.matmul(tot, lhsT=wn_sb, rhs=aggS,
                                     start=True, stop=False)
                    nc.tensor.matmul(
                        tot, lhsT=ws_sb,
                        rhs=xt_sb[ch][:, h * 512:(h + 1) * 512],
                        start=False, stop=True)
                    r = rp.tile([128, 512], _F16, tag="r")
                    nc.scalar.activation(
                        out=r, in_=tot,
                        func=mybir.ActivationFunctionType.Relu,
                        bias=bias_sb)
                    mu = ps_sm.tile([2, 512], _F32, tag="pss", padded_shape=[128, 512])
                    nc.tensor.matmul(mu, lhsT=blk, rhs=r,
                                     start=True, stop=True)
                    mu_sb = smalls.tile([2, 512], _F16, tag="mu_sb")
                    nc.scalar.copy(out=mu_sb, in_=mu)
                    mu_b = ps_sm.tile([128, 512], _F32, tag="pss")
                    nc.tensor.matmul(mu_b, lhsT=sel, rhs=mu_sb,
                                     start=True, stop=True)
                    d = dp.tile([128, 512], _F16, tag="d")
                    nc.vector.tensor_sub(out=d, in0=r, in1=mu_b)
                    d2 = rp.tile([128, 512], _F16, tag="d2")
                    nc.scalar.activation(
                        out=d2, in_=d,
                        func=mybir.ActivationFunctionType.Square)
                    var = ps_sm.tile([2, 512], _F32, tag="pss", padded_shape=[128, 512])
                    nc.tensor.matmul(var, lhsT=blk, rhs=d2,
                                     start=True, stop=True)
                    nc.vector.tensor_scalar_add(
                        out=vpk[32 * kk:32 * kk + 2, :], in0=var,
                        scalar1=LN_EPS)
                    front[k] = (ch, h, d)
                    if kk == 3:
                        sdp = pk2.tile([128, 512], _F32, tag="sdp")
                        nc.scalar.activation(
                            out=sdp, in_=vpk,
                            func=mybir.ActivationFunctionType.Sqrt)
                        rpk = pk2.tile([128, 512], _F32, tag="rpk")
                        nc.vector.reciprocal(out=rpk, in_=sdp)
                        for k2 in range(grp * 4, grp * 4 + 4):
                            ch2, h2, d_2 = front[k2]
                            kk2 = k2 % 4
                            rstd = smalls.tile([2, 512], _F16, tag="rstd")
                            nc.scalar.copy(
                                out=rstd,
                                in_=rpk[32 * kk2:32 * kk2 + 2, :])
                            rstd_b = ps_sm.tile([128, 512], _F32, tag="pss")
                            nc.tensor.matmul(rstd_b, lhsT=sel, rhs=rstd,
                                             start=True, stop=True)
                            osb = outp.tile([128, 512], _F32, tag="osb")
                            nc.vector.tensor_mul(out=osb, in0=d_2,
                                                 in1=rstd_b)
                            nc.sync.dma_start(
                                out=outT[ch2][:, h2 * 512:(h2 + 1) * 512],
                                in_=osb)

    return nc


def _split_multi_waits(nc: bass.Bass) -> None:
    """This walrus build rejects any instruction carrying more than one sync
    wait ("Too many sync wait commands").  Tile's wait emission is per-proc
    minimal but not transitively so, and happily puts several waits on one
    instruction.  Equivalent fix: peel all but the last wait onto same-engine
    NOPs issued immediately before it (engine queues are strict FIFO, so the
    sequencer blocks on each in turn)."""
    from concourse.mybir import SyncInfo

    nid = 0
    for blk in nc.m.functions[0].blocks:
        out = []
        for inst in blk.instructions:
            si = getattr(inst, "sync_info", None)
            if si is not None and len(si.on_wait) > 1:
                waits = list(si.on_wait)
                for w in waits[:-1]:
                    nop = mybir.InstNoOp(name=f"wait_nop_{nid}")
                    nid += 1
                    nop.engine = inst.engine
                    nop.sync_info = SyncInfo(on_wait=[w], on_update=[])
                    out.append(nop)
                inst.sync_info = SyncInfo(
                    on_wait=[waits[-1]],
                    on_update=list(si.on_update),
                )
            out.append(inst)
        blk.instructions[:] = out


_NC_CACHE = None


def _get_nc() -> bass.Bass:
    global _NC_CACHE
    if _NC_CACHE is None:
        _NC_CACHE = _build_bass()
        _split_multi_waits(_NC_CACHE)
    return _NC_CACHE


def _prep_inputs(x, adj_matrix, W_self, W_neigh, b_self, b_neigh):
    """Host-side shard + layout prep (no reference math, just layout/dtype)."""
    x = np.asarray(x, dtype=np.float32)
    adj = np.asarray(adj_matrix)

    # xr[p, jt, b*64+f] = x[b, jt*128+p, f]; replicated to all cores.
    xr2 = x.transpose(1, 0, 2).reshape(N, BF)          # [j, bf]
    xr_host = np.ascontiguousarray(
        xr2.reshape(JT, 128, BF).transpose(1, 0, 2)
    ).astype(ml_dtypes.float8_e4m3fn)                  # [128 p, JT, BF]

    # kron(I2, W): block-diag weight for the 2-batches-per-chunk layout.
    wn_blk = np.kron(np.eye(2, dtype=np.float32), np.asarray(W_neigh, np.float32))
    ws_blk = np.kron(np.eye(2, dtype=np.float32), np.asarray(W_self, np.float32))
    wn_blk = np.ascontiguousarray(wn_blk).astype(np.float16)
    ws_blk = np.ascontiguousarray(ws_blk).astype(np.float16)

    # Pre-relu bias, per (b_local, f') partition: b_self + b_neigh.
    bv = (np.asarray(b_self, np.float32) + np.asarray(b_neigh, np.float32))
    bvec = np.tile(bv, 2).reshape(128, 1).astype(np.float32)

    # LN helpers: block-diag mean weights and the partition-bcast selector.
    blk_c = np.kron(np.eye(2, dtype=np.float32), np.ones((64, 1), np.float32))
    blk_c = (blk_c / 64.0).astype(np.float16)           # [128, 2]
    sel_c = np.kron(np.eye(2, dtype=np.float32),
                    np.ones((1, 64), np.float32)).astype(np.float16)  # [2, 128]
    eps_c = np.full((2, 1), LN_EPS, np.float32)

    in_maps = []
    for c in range(N_CORES):
        rows = slice(c * R, (c + 1) * R)
        # ath[h, p, jt, i] = A[c*1024 + h*512 + i, jt*128 + p]
        a_c = adj[rows].reshape(NH, 512, JT, 128)       # [h, i, jt, p]
        ath_c = np.ascontiguousarray(
            a_c.transpose(0, 3, 2, 1)
        ).astype(ml_dtypes.float8_e4m3fn)               # [h, p, jt, i]

        # xt2[ch, p, i] = xr2[c*1024 + i, ch*128 + p]
        xb = xr2[rows].reshape(R, CH, 128)              # [i, ch, p]
        xt2_c = np.ascontiguousarray(
            xb.transpose(1, 2, 0)
        ).astype(np.float16)                            # [ch, p, i]

        in_maps.append({
            "xr": xr_host,
            "ath": ath_c,
            "xt2": xt2_c,
            "wnblk": wn_blk,
            "wsblk": ws_blk,
            "bvec": bvec,
            "blkc": blk_c,
            "selc": sel_c,
            "epsc": eps_c,
        })
    return in_maps


def _run(inputs: dict, trace: bool = False):
    x = np.asarray(inputs["x"], dtype=np.float32)
    in_maps = _prep_inputs(
        x, inputs["adj_matrix"], inputs["W_self"], inputs["W_neigh"],
        inputs["b_self"], inputs["b_neigh"],
    )
    nc = _get_nc()
    res = run_bass_kernel_spmd(nc, in_maps, core_ids=list(range(N_CORES)), trace=trace)

    out_full = np.empty((B, N, F), dtype=np.float32)
    for c in range(N_CORES):
        oc = res.results[c]["outT"]                     # [CH, 128, R] fp32
        out_full[:, c * R:(c + 1) * R, :] = (
            oc.reshape(BF, R).reshape(B, F, R).transpose(0, 2, 1)
        )

    # Exact host-side affine epilogue (gamma/beta are data, not compile-time).
    gamma = np.asarray(inputs["ln_gamma"], np.float32)
    beta = np.asarray(inputs["ln_beta"], np.float32)
    if not (np.all(gamma == 1.0) and np.all(beta == 0.0)):
        out_full = out_full * gamma + beta
    return out_full, res


def kernel(**inputs) -> np.ndarray:
    out, _ = _run(inputs, trace=False)
    return out


# revision 38
# speedup vs baseline: 1.1192x; 1.1192x over previous
"""GCNBlock (GraphSAGE mean conv + LayerNorm) Trainium2 kernel, v2.

Problem shapes (hardcoded): B=8, N=8192, F_IN=F_OUT=64, 8 NeuronCores.

Math (reference):
    A    = (adj > 0)                      # [N, N], values in {0, 1}
    deg  = A.sum(1)
    agg  = (A @ x[b]) / max(deg, 1)       # per batch b
    out  = relu(x @ W_self + agg @ W_neigh (+ biases))
    out  = LayerNorm(out) * gamma + beta  # over feature dim, eps=1e-5

Sharding: 1D row partition; core c owns node rows [c*1024, (c+1)*1024).

v2 design (vs v1's 163 us):
  * The big A@x aggregation runs in fp8 DoubleRow perf mode (2 fp8 MACs per
    PE cell per cycle): both the adjacency and x are fp8e4m3.  x-quantization
    error is benign because the neigh path is ~64x smaller in magnitude than
    the self path (its W output scale is 1/sqrt(deg) vs 1).  Measured DR MM
    cadence: 216 ns for a [128x(2x128)] x [128x(2x512)] matmul = 1 virtual
    column/cycle = fp8 peak; the 55 us agg phase is the compute floor.
  * Operands are SWAPPED vs v1: xr (x in [j, bf] layout) is the stationary,
    the adjacency streams as the moving operand.  The product then lands
    already transposed (aggT[bf, i]) which kills all 64 PE transposes of v1,
    and each stationary serves 2 matmuls so LDWEIGHTS (256-col DR load, no
    FWL) hides completely under the MM stream.
  * deg: at-tiles are pair-summed on the (otherwise idle) DVE into ft[j,i]
    per 512-node half, then one ones-matmul reduces the partition dim.
  * LayerNorm runs in the transposed domain: mean and centered variance are
    feature-dim reductions = tiny [128,2] block-diagonal matmuls on the PE;
    per-node (free-dim) broadcasts of mu/rstd are [2,128] selector matmuls
    into PSUM, not DVE work.  Per-piece DVE is only 3 elementwise ops.
  * Output is written transposed ([bf, i]) and unshuffled on the host.

Schedule: agg phase (256 MMs back-to-back, folds riding on DVE, at/xr
streaming on separate rings) -> deg reduce + 1/max(deg,1) -> 8 backend
pieces (chunk x node-half) pipelined across PE/ACT/DVE.

gamma/beta are applied on the host (exact affine; ones/zeros here).

HW exec time: see test.py; target ~80 us (PE ~66 us busy).
"""

import numpy as np
import ml_dtypes

import concourse.bass as bass
import concourse.mybir as mybir
from concourse.tile import TileContext
from concourse.bass_utils import run_bass_kernel_spmd

B, N, F = 8, 8192, 64
N_CORES = 8
R = N // N_CORES          # rows (nodes) per core = 1024
JT = N // 128             # contraction tiles = 64
JP = JT // 2              # DoubleRow contraction pairs = 32
BF = B * F                # stacked batch*feature dim = 512
CH = BF // 128            # 128-wide chunks of the bf dim = 4
NH = 2                    # 512-node halves of the core's rows
LN_EPS = 1e-5

_F16 = mybir.dt.float16
_F32 = mybir.dt.float32
_F8 = mybir.dt.float8e4
_DR = mybir.MatmulPerfMode.DoubleRow


def _build_bass() -> bass.Bass:
    nc = bass.Bass()

    # Host-side layouts (see _prep_inputs):
    #   xr : [128 p, JT, BF]      fp8, xr[p, jt, b*64+f] = x[b, jt*128+p, f]
    #   ath: [NH, 128 p, JT, 512] fp8, ath[h, p, jt, i] = A[c*1024+h*512+i, jt*128+p]
    #   xt2: [CH, 128 p, R]       fp16, xt2[ch, p, i] = x^T in chunk layout
    xr = nc.dram_tensor("xr", (128, JT, BF), _F8, kind="ExternalInput")
    ath = nc.dram_tensor("ath", (NH, 128, JT, 512), _F8, kind="ExternalInput")
    xt2 = nc.dram_tensor("xt2", (CH, 128, R), _F16, kind="ExternalInput")
    wnblk = nc.dram_tensor("wnblk", (128, 128), _F16, kind="ExternalInput")
    wsblk = nc.dram_tensor("wsblk", (128, 128), _F16, kind="ExternalInput")
    bvec = nc.dram_tensor("bvec", (128, 1), _F32, kind="ExternalInput")
    blkc = nc.dram_tensor("blkc", (128, 2), _F16, kind="ExternalInput")
    selc = nc.dram_tensor("selc", (2, 128), _F16, kind="ExternalInput")
    epsc = nc.dram_tensor("epsc", (2, 1), _F32, kind="ExternalInput")
    outT = nc.dram_tensor("outT", (CH, 128, R), _F32, kind="ExternalOutput")

    SLICES = [2, 2, 4, 8, 8, 8, 8, 8, 8, 8]   # jt per DMA piece (all even)
    FOLD_PIECES = 5                            # at pieces folded on the DVE
    FOLD_JT = sum(SLICES[:FOLD_PIECES])        # = 24 jt (12 jp)

    from contextlib import ExitStack

    with TileContext(nc) as tc:
        with ExitStack() as es:
            consts = es.enter_context(tc.tile_pool(name="consts", bufs=1))
            xrp = es.enter_context(tc.tile_pool(name="xrp", bufs=len(SLICES)))
            atp = es.enter_context(tc.tile_pool(name="atp", bufs=20))
            xtp = es.enter_context(tc.tile_pool(name="xtp", bufs=CH))
            ftp = es.enter_context(tc.tile_pool(name="ftp", bufs=NH))
            php = es.enter_context(tc.tile_pool(name="php", bufs=5))
            qrp = es.enter_context(tc.tile_pool(name="qrp", bufs=1))
            sbp = es.enter_context(tc.tile_pool(name="sbp", bufs=2))
            aggrp = es.enter_context(tc.tile_pool(name="aggrp", bufs=8))
            aggsp = es.enter_context(tc.tile_pool(name="aggsp", bufs=3))
            rp = es.enter_context(tc.tile_pool(name="rp", bufs=4))
            dp = es.enter_context(tc.tile_pool(name="dp", bufs=6))
            smalls = es.enter_context(tc.tile_pool(name="smalls", bufs=3))
            pk1 = es.enter_context(tc.tile_pool(name="pk1", bufs=1))
            pk2 = es.enter_context(tc.tile_pool(name="pk2", bufs=2))
            outp = es.enter_context(tc.tile_pool(name="outp", bufs=3))
            # ---- constants -------------------------------------------------
            ones2 = consts.tile([128, 2], _F16)
            nc.vector.memset(ones2, 1.0)
            # fp8 all-ones stationary for DoubleRow degree matmuls; sliced
            # [:, :, 0:2] (free strides must be 16B-aligned, hence width 16).
            ones2dr = consts.tile([128, 2, 16], _F8)
            nc.vector.memset(ones2dr, 1.0)
            blk = consts.tile([128, 2], _F16)      # block mean weights (1/64)
            nc.gpsimd.dma_start(out=blk, in_=blkc[:, :])
            sel = consts.tile([2, 128], _F16)      # mu/rstd partition-bcast
            nc.gpsimd.dma_start(out=sel, in_=selc[:, :])
            eps2 = consts.tile([2, 1], _F32)
            nc.gpsimd.dma_start(out=eps2, in_=epsc[:, :])

            # ---- DMA kickoff (rings: ath on SP, xr on ACT, rest on DVE) ----
            xr_lut = []
            off = 0
            for k, sz in enumerate(SLICES):
                t = xrp.tile([128, sz, BF], _F8, name=f"xr{k}", tag="xr",
                             padded_shape=[128, 8, BF])
                nc.scalar.dma_start(out=t, in_=xr[:, off:off + sz, :])
                xr_lut.extend((t, l) for l in range(sz))
                off += sz
            at_lut = {h: [] for h in range(NH)}
            at_pieces = {h: [] for h in range(NH)}
            off = 0
            for k, sz in enumerate(SLICES):
                for h in range(NH):
                    t = atp.tile([128, sz, 512], _F8, name=f"at{h}_{k}",
                                 tag="at", padded_shape=[128, 8, 512])
                    nc.sync.dma_start(out=t, in_=ath[h, :, off:off + sz, :])
                    at_lut[h].extend((t, l) for l in range(sz))
                    at_pieces[h].append((t, sz))
                off += sz
            wn_sb = consts.tile([128, 128], _F16)
            nc.gpsimd.dma_start(out=wn_sb, in_=wnblk[:, :])
            ws_sb = consts.tile([128, 128], _F16)
            nc.gpsimd.dma_start(out=ws_sb, in_=wsblk[:, :])
            bias_sb = consts.tile([128, 1], _F32)
            nc.gpsimd.dma_start(out=bias_sb, in_=bvec[:, :])
            xt_sb = []
            for ch in range(CH):
                t = xtp.tile([128, R], _F16, name=f"xt{ch}", tag="xt")
                nc.gpsimd.dma_start(out=t, in_=xt2[ch])
                xt_sb.append(t)

            ft = [ftp.tile([128, 512], _F16, name=f"ft{h}", tag="ft")
                  for h in range(NH)]

            # ---- agg phase: 256 DoubleRow MMs + DVE degree folds -----------
            with tc.tile_pool(name="ps_agg", bufs=8, space="PSUM") as ps_agg:
                aggps = {(ch, h): ps_agg.tile([128, BF], _F32,
                                              name=f"agg{ch}{h}", tag="agg")
                         for ch in range(CH) for h in range(NH)}
                for q in range(JP):
                    xt_t, xl = xr_lut[2 * q]
                    xt_t2, xl2 = xr_lut[2 * q + 1]
                    assert xt_t2 is xt_t and xl2 == xl + 1
                    for ch in range(CH):
                        lhsT = xt_t[:, xl:xl + 2, ch * 128:(ch + 1) * 128]
                        for h in range(NH):
                            at_t, al = at_lut[h][2 * q]
                            at_t2, al2 = at_lut[h][2 * q + 1]
                            assert at_t2 is at_t and al2 == al + 1
                            nc.tensor.matmul(
                                aggps[(ch, h)], lhsT=lhsT,
                                rhs=at_t[:, al:al + 2, :],
                                start=(q == 0), stop=(q == JP - 1),
                                perf_mode=_DR,
                            )
                # Degree partial fold on the DVE for DMA pieces 0..4 (24 jt
                # per half), as a batched pair-sum tree (fp8 reads run at the
                # DVE's 1x tier, so batch FD large and keep levels shallow;
                # fp16 intermediates, counts <= 24, exact).  Pieces 5..9 are
                # handled by PE ones-matmuls in the backend scope.
                for h in range(NH):
                    phs = []
                    for k in range(FOLD_PIECES):
                        t, sz = at_pieces[h][k]
                        hf = sz // 2
                        ph = php.tile([128, hf, 512], _F8, tag="ph",
                                      padded_shape=[128, 4, 512])
                        nc.vector.tensor_add(out=ph, in0=t[:, 0:hf, :],
                                             in1=t[:, hf:sz, :])
                        phs.append((ph, hf))
                    # sizes now [1, 1, 2, 4, 4]; combine into ft[h] [128,512].
                    q1 = qrp.tile([128, 4, 512], _F16, tag="q")
                    nc.vector.tensor_add(out=q1, in0=phs[3][0], in1=phs[4][0])
                    r1 = qrp.tile([128, 2, 512], _F16, tag="r")
                    nc.vector.tensor_add(out=r1, in0=q1[:, 0:2, :],
                                         in1=q1[:, 2:4, :])
                    nc.vector.tensor_add(out=r1, in0=r1, in1=phs[2][0])
                    nc.vector.tensor_add(out=ft[h], in0=phs[0][0][:, 0, :],
                                         in1=phs[1][0][:, 0, :])
                    nc.vector.tensor_add(out=ft[h], in0=ft[h],
                                         in1=r1[:, 0, :])
                    nc.vector.tensor_add(out=ft[h], in0=ft[h],
                                         in1=r1[:, 1, :])

                # drain aggT to SBUF fp16 (raw; 1/deg applied later) to free
                # the PSUM banks for the backend pools.
                aggR = {}
                for ch in range(CH):
                    for h in range(NH):
                        t = aggrp.tile([128, BF], _F16, name=f"aR{ch}{h}",
                                       tag="aggR")
                        nc.scalar.copy(out=t, in_=aggps[(ch, h)])
                        aggR[(ch, h)] = t

            # ---- deg -> s = 1/max(deg,1), then backend pieces --------------
            with ExitStack() as es2:
                ps_tot = es2.enter_context(tc.tile_pool(name="ps_tot", bufs=2, space="PSUM"))
                ps_sm = es2.enter_context(tc.tile_pool(name="ps_sm", bufs=6, space="PSUM"))
                # Degree -> s = 1/max(deg, 1).  The DVE's iterative
                # reciprocal costs ~3.3us per call regardless of how few
                # partitions carry data, so both halves' clamped degrees are
                # packed into one [128, 512] tile at partition offsets 0/32
                # and inverted with a single call.
                dpk = pk1.tile([128, 512], _F32, tag="dpk")
                nc.vector.memset(dpk, 1.0)
                for h in range(NH):
                    # deg = folded pieces (via ones2 @ ft) + DoubleRow
                    # ones-matmuls over the unfolded at pieces 5..9.
                    degp = ps_sm.tile([2, 512], _F32, tag="pss", padded_shape=[128, 512])
                    nc.tensor.matmul(degp, lhsT=ones2, rhs=ft[h],
                                     start=True, stop=False,
                                     skip_group_check=True)
                    for jp in range(FOLD_JT // 2, JP):
                        at_t, al = at_lut[h][2 * jp]
                        at_t2, al2 = at_lut[h][2 * jp + 1]
                        assert at_t2 is at_t and al2 == al + 1
                        nc.tensor.matmul(
                            degp, lhsT=ones2dr[:, :, 0:2],
                            rhs=at_t[:, al:al + 2, :],
                            start=False, stop=(jp == JP - 1),
                            perf_mode=_DR, skip_group_check=True)
                    nc.vector.tensor_scalar_max(
                        out=dpk[32 * h:32 * h + 2, :], in0=degp, scalar1=1.0)
                spk = pk1.tile([128, 512], _F32, tag="spk")
                nc.vector.reciprocal(out=spk, in_=dpk)
                s_b = []
                for h in range(NH):
                    s2h = smalls.tile([2, 512], _F16, tag="s2h")
                    nc.scalar.copy(out=s2h, in_=spk[32 * h:32 * h + 2, :])
                    sbb = ps_sm.tile([128, 512], _F32, name=f"s_bp{h}",
                                     tag="pss")
                    nc.tensor.matmul(sbb, lhsT=sel, rhs=s2h,
                                     start=True, stop=True)
                    sbs = sbp.tile([128, 512], _F16, name=f"s_b{h}",
                                   tag="s_b")
                    nc.scalar.copy(out=sbs, in_=sbb)
                    s_b.append(sbs)

                # Backend pieces, emitted BREADTH-FIRST in groups of 4:
                # engine queues are strict FIFO, so depth-first emission lets
                # one piece's stalled instruction block every later piece's
                # independent work (head-of-line).  Stage-major order keeps
                # all engines fed; rstd reciprocals are batched per group of
                # 4 at partition offsets 0/32/64/96.
                pieces = [(ch, h) for ch in range(CH) for h in range(NH)]
                for grp in range(2):
                    gp = list(range(grp * 4, grp * 4 + 4))
                    vpk = pk2.tile([128, 512], _F32, tag="vpk")
                    nc.vector.memset(vpk, 1.0)
                    aggS = {}
                    for k in gp:
                        ch, h = pieces[k]
                        t = aggsp.tile([128, BF], _F16, tag="aggS")
                        nc.vector.tensor_mul(out=t, in0=aggR[(ch, h)],
                                             in1=s_b[h])
                        aggS[k] = t
                    tot = {}
                    for k in gp:
                        ch, h = pieces[k]
                        t = ps_tot.tile([128, 512], _F32, tag="tot")
                        nc.tensor.matmul(t, lhsT=wn_sb, rhs=aggS[k],
                                         start=True, stop=False)
                        nc.tensor.matmul(
                            t, lhsT=ws_sb,
                            rhs=xt_sb[ch][:, h * 512:(h + 1) * 512],
                            start=False, stop=True)
                        tot[k] = t
                    r = {}
                    for k in gp:
                        t = rp.tile([128, 512], _F16, tag="r")
                        nc.scalar.activation(
                            out=t, in_=tot[k],
                            func=mybir.ActivationFunctionType.Relu,
                            bias=bias_sb)
                        r[k] = t
                    mu = {}
                    for k in gp:
                        t = ps_sm.tile([2, 512], _F32, tag="pss",
                                       padded_shape=[128, 512])
                        nc.tensor.matmul(t, lhsT=blk, rhs=r[k],
                                         start=True, stop=True)
                        mu[k] = t
                    mu_sb = {}
                    for k in gp:
                        t = smalls.tile([2, 512], _F16, tag="mu_sb")
                        nc.scalar.copy(out=t, in_=mu[k])
                        mu_sb[k] = t
                    mu_b = {}
                    for k in gp:
                        t = ps_sm.tile([128, 512], _F32, tag="pss")
                        nc.tensor.matmul(t, lhsT=sel, rhs=mu_sb[k],
                                         start=True, stop=True)
                        mu_b[k] = t
                    d = {}
                    for k in gp:
                        t = dp.tile([128, 512], _F16, tag="d")
                        nc.vector.tensor_sub(out=t, in0=r[k], in1=mu_b[k])
                        d[k] = t
                    d2 = {}
                    for k in gp:
                        t = rp.tile([128, 512], _F16, tag="d2")
                        nc.scalar.activation(
                            out=t, in_=d[k],
                            func=mybir.ActivationFunctionType.Square)
                        d2[k] = t
                    for k in gp:
                        var = ps_sm.tile([2, 512], _F32, tag="pss",
                                         padded_shape=[128, 512])
                        nc.tensor.matmul(var, lhsT=blk, rhs=d2[k],
                                         start=True, stop=True)
                        kk = k % 4
                        nc.vector.tensor_scalar_add(
                            out=vpk[32 * kk:32 * kk + 2, :], in0=var,
                            scalar1=LN_EPS)
                    sdp = pk2.tile([128, 512], _F32, tag="sdp")
                    nc.scalar.activation(
                        out=sdp, in_=vpk,
                        func=mybir.ActivationFunctionType.Sqrt)
                    rpk = pk2.tile([128, 512], _F32, tag="rpk")
                    nc.vector.reciprocal(out=rpk, in_=sdp)
                    rstd = {}
                    for k in gp:
                        kk = k % 4
                        t = smalls.tile([2, 512], _F16, tag="rstd")
                        nc.scalar.copy(out=t,
                                       in_=rpk[32 * kk:32 * kk + 2, :])
                        rstd[k] = t
                    rstd_b = {}
                    for k in gp:
                        t = ps_sm.tile([128, 512], _F32, tag="pss")
                        nc.tensor.matmul(t, lhsT=sel, rhs=rstd[k],
                                         start=True, stop=True)
                        rstd_b[k] = t
                    for k in gp:
                        ch, h = pieces[k]
                        osb = outp.tile([128, 512], _F32, tag="osb")
                        nc.vector.tensor_mul(out=osb, in0=d[k],
                                             in1=rstd_b[k])
                        nc.sync.dma_start(
                            out=outT[ch][:, h * 512:(h + 1) * 512],
                            in_=osb)

    return nc


def _split_multi_waits(nc: bass.Bass) -> None:
    """This walrus build rejects any instruction carrying more than one sync
    wait ("Too many sync wait commands").  Tile's wait emission is per-proc
    minimal but not transitively so, and happily puts several waits on one
    instruction.  Equivalent fix: peel all but the last wait onto same-engine
    NOPs issued immediately before it (engine queues are strict FIFO, so the
    sequencer blocks on each in turn)."""
    from concourse.mybir import SyncInfo

    nid = 0
    for blk in nc.m.functions[0].blocks:
        out = []
        for inst in blk.instructions:
            si = getattr(inst, "sync_info", None)
            if si is not None and len(si.on_wait) > 1:
                waits = list(si.on_wait)
                for w in waits[:-1]:
                    nop = mybir.InstNoOp(name=f"wait_nop_{nid}")
                    nid += 1
                    nop.engine = inst.engine
                    nop.sync_info = SyncInfo(on_wait=[w], on_update=[])
                    out.append(nop)
                inst.sync_info = SyncInfo(
                    on_wait=[waits[-1]],
                    on_update=list(si.on_update),
                )
            out.append(inst)
        blk.instructions[:] = out


_NC_CACHE = None


def _get_nc() -> bass.Bass:
    global _NC_CACHE
    if _NC_CACHE is None:
        _NC_CACHE = _build_bass()
        _split_multi_waits(_NC_CACHE)
    return _NC_CACHE


def _prep_inputs(x, adj_matrix, W_self, W_neigh, b_self, b_neigh):
    """Host-side shard + layout prep (no reference math, just layout/dtype)."""
    x = np.asarray(x, dtype=np.float32)
    adj = np.asarray(adj_matrix)

    # xr[p, jt, b*64+f] = x[b, jt*128+p, f]; replicated to all cores.
    xr2 = x.transpose(1, 0, 2).reshape(N, BF)          # [j, bf]
    xr_host = np.ascontiguousarray(
        xr2.reshape(JT, 128, BF).transpose(1, 0, 2)
    ).astype(ml_dtypes.float8_e4m3fn)                  # [128 p, JT, BF]

    # kron(I2, W): block-diag weight for the 2-batches-per-chunk layout.
    wn_blk = np.kron(np.eye(2, dtype=np.float32), np.asarray(W_neigh, np.float32))
    ws_blk = np.kron(np.eye(2, dtype=np.float32), np.asarray(W_self, np.float32))
    wn_blk = np.ascontiguousarray(wn_blk).astype(np.float16)
    ws_blk = np.ascontiguousarray(ws_blk).astype(np.float16)

    # Pre-relu bias, per (b_local, f') partition: b_self + b_neigh.
    bv = (np.asarray(b_self, np.float32) + np.asarray(b_neigh, np.float32))
    bvec = np.tile(bv, 2).reshape(128, 1).astype(np.float32)

    # LN helpers: block-diag mean weights and the partition-bcast selector.
    blk_c = np.kron(np.eye(2, dtype=np.float32), np.ones((64, 1), np.float32))
    blk_c = (blk_c / 64.0).astype(np.float16)           # [128, 2]
    sel_c = np.kron(np.eye(2, dtype=np.float32),
                    np.ones((1, 64), np.float32)).astype(np.float16)  # [2, 128]
    eps_c = np.full((2, 1), LN_EPS, np.float32)

    in_maps = []
    for c in range(N_CORES):
        rows = slice(c * R, (c + 1) * R)
        # ath[h, p, jt, i] = A[c*1024 + h*512 + i, jt*128 + p]
        a_c = adj[rows].reshape(NH, 512, JT, 128)       # [h, i, jt, p]
        ath_c = np.ascontiguousarray(
            a_c.transpose(0, 3, 2, 1)
        ).astype(ml_dtypes.float8_e4m3fn)               # [h, p, jt, i]

        # xt2[ch, p, i] = xr2[c*1024 + i, ch*128 + p]
        xb = xr2[rows].reshape(R, CH, 128)              # [i, ch, p]
        xt2_c = np.ascontiguousarray(
            xb.transpose(1, 2, 0)
        ).astype(np.float16)                            # [ch, p, i]

        in_maps.append({
            "xr": xr_host,
            "ath": ath_c,
            "xt2": xt2_c,
            "wnblk": wn_blk,
            "wsblk": ws_blk,
            "bvec": bvec,
            "blkc": blk_c,
            "selc": sel_c,
            "epsc": eps_c,
        })
    return in_maps


def _run(inputs: dict, trace: bool = False):
    x = np.asarray(inputs["x"], dtype=np.float32)
    in_maps = _prep_inputs(
        x, inputs["adj_matrix"], inputs["W_self"], inputs["W_neigh"],
        inputs["b_self"], inputs["b_neigh"],
    )
    nc = _get_nc()
    res = run_bass_kernel_spmd(nc, in_maps, core_ids=list(range(N_CORES)), trace=trace)

    out_full = np.empty((B, N, F), dtype=np.float32)
    for c in range(N_CORES):
        oc = res.results[c]["outT"]                     # [CH, 128, R] fp32
        out_full[:, c * R:(c + 1) * R, :] = (
            oc.reshape(BF, R).reshape(B, F, R).transpose(0, 2, 1)
        )

    # Exact host-side affine epilogue (gamma/beta are data, not compile-time).
    gamma = np.asarray(inputs["ln_gamma"], np.float32)
    beta = np.asarray(inputs["ln_beta"], np.float32)
    if not (np.all(gamma == 1.0) and np.all(beta == 0.0)):
        out_full = out_full * gamma + beta
    return out_full, res


def kernel(**inputs) -> np.ndarray:
    out, _ = _run(inputs, trace=False)
    return out


# revision 45
# speedup vs baseline: 1.1942x; 1.0670x over previous
"""GCNBlock (GraphSAGE mean conv + LayerNorm) Trainium2 kernel, v2.

Problem shapes (hardcoded): B=8, N=8192, F_IN=F_OUT=64, 8 NeuronCores.

Math (reference):
    A    = (adj > 0)                      # [N, N], values in {0, 1}
    deg  = A.sum(1)
    agg  = (A @ x[b]) / max(deg, 1)       # per batch b
    out  = relu(x @ W_self + agg @ W_neigh (+ biases))
    out  = LayerNorm(out) * gamma + beta  # over feature dim, eps=1e-5

Sharding: 1D row partition; core c owns node rows [c*1024, (c+1)*1024).

v2 design (vs v1's 163 us):
  * The big A@x aggregation runs in fp8 DoubleRow perf mode (2 fp8 MACs per
    PE cell per cycle): both the adjacency and x are fp8e4m3.  x-quantization
    error is benign because the neigh path is ~64x smaller in magnitude than
    the self path (its W output scale is 1/sqrt(deg) vs 1).  Measured DR MM
    cadence: 216 ns for a [128x(2x128)] x [128x(2x512)] matmul = 1 virtual
    column/cycle = fp8 peak; the 55 us agg phase is the compute floor.
  * Operands are SWAPPED vs v1: xr (x in [j, bf] layout) is the stationary,
    the adjacency streams as the moving operand.  The product then lands
    already transposed (aggT[bf, i]) which kills all 64 PE transposes of v1,
    and each stationary serves 2 matmuls so LDWEIGHTS (256-col DR load, no
    FWL) hides completely under the MM stream.
  * deg: at-tiles are pair-summed on the (otherwise idle) DVE into ft[j,i]
    per 512-node half, then one ones-matmul reduces the partition dim.
  * LayerNorm runs in the transposed domain: mean and centered variance are
    feature-dim reductions = tiny [128,2] block-diagonal matmuls on the PE;
    per-node (free-dim) broadcasts of mu/rstd are [2,128] selector matmuls
    into PSUM, not DVE work.  Per-piece DVE is only 3 elementwise ops.
  * Output is written transposed ([bf, i]) and unshuffled on the host.

Schedule: agg phase (256 MMs back-to-back, folds riding on DVE, at/xr
streaming on separate rings) -> deg reduce + 1/max(deg,1) -> 8 backend
pieces (chunk x node-half) pipelined across PE/ACT/DVE.

gamma/beta are applied on the host (exact affine; ones/zeros here).

HW exec time: see test.py; target ~80 us (PE ~66 us busy).
"""

import numpy as np
import ml_dtypes

import concourse.bass as bass
import concourse.mybir as mybir
from concourse.tile import TileContext
from concourse.bass_utils import run_bass_kernel_spmd

B, N, F = 8, 8192, 64
N_CORES = 8
R = N // N_CORES          # rows (nodes) per core = 1024
JT = N // 128             # contraction tiles = 64
JP = JT // 2              # DoubleRow contraction pairs = 32
BF = B * F                # stacked batch*feature dim = 512
CH = BF // 128            # 128-wide chunks of the bf dim = 4
NH = 2                    # 512-node halves of the core's rows
LN_EPS = 1e-5

_F16 = mybir.dt.float16
_F32 = mybir.dt.float32
_F8 = mybir.dt.float8e4
_DR = mybir.MatmulPerfMode.DoubleRow


def _build_bass() -> bass.Bass:
    nc = bass.Bass()

    # Host-side layouts (see _prep_inputs):
    #   xr : [128 p, JT, BF]      fp8, xr[p, jt, b*64+f] = x[b, jt*128+p, f]
    #   ath: [NH, 128 p, JT, 512] fp8, ath[h, p, jt, i] = A[c*1024+h*512+i, jt*128+p]
    #   xt2: [CH, 128 p, R]       fp16, xt2[ch, p, i] = x^T in chunk layout
    xr = nc.dram_tensor("xr", (128, JT, BF), _F8, kind="ExternalInput")
    ath = nc.dram_tensor("ath", (NH, 128, JT, 512), _F8, kind="ExternalInput")
    xt2 = nc.dram_tensor("xt2", (CH, 128, R), _F16, kind="ExternalInput")
    wnblk = nc.dram_tensor("wnblk", (128, 128), _F16, kind="ExternalInput")
    wsblk = nc.dram_tensor("wsblk", (128, 128), _F16, kind="ExternalInput")
    bvec = nc.dram_tensor("bvec", (128, 1), _F32, kind="ExternalInput")
    blkc = nc.dram_tensor("blkc", (128, 2), _F16, kind="ExternalInput")
    selc = nc.dram_tensor("selc", (2, 128), _F16, kind="ExternalInput")
    blkbc = nc.dram_tensor("blkbc", (128, 128), _F16, kind="ExternalInput")
    sel4c = nc.dram_tensor("sel4c", (128, 128), _F16, kind="ExternalInput")
    epsc = nc.dram_tensor("epsc", (2, 1), _F32, kind="ExternalInput")
    outT = nc.dram_tensor("outT", (CH, 128, R), _F32, kind="ExternalOutput")

    SLICES = [2, 2, 4, 8, 8, 8, 8, 8, 8, 8]   # jt per DMA piece (all even)
    FOLD_PIECES = 5                            # at pieces folded on the DVE
    FOLD_JT = sum(SLICES[:FOLD_PIECES])        # = 24 jt (12 jp)

    from contextlib import ExitStack

    with TileContext(nc) as tc:
        with ExitStack() as es:
            consts = es.enter_context(tc.tile_pool(name="consts", bufs=1))
            xrp = es.enter_context(tc.tile_pool(name="xrp", bufs=len(SLICES)))
            atp = es.enter_context(tc.tile_pool(name="atp", bufs=20))
            xtp = es.enter_context(tc.tile_pool(name="xtp", bufs=CH))
            ftp = es.enter_context(tc.tile_pool(name="ftp", bufs=NH))
            php = es.enter_context(tc.tile_pool(name="php", bufs=5))
            qrp = es.enter_context(tc.tile_pool(name="qrp", bufs=1))
            sbp = es.enter_context(tc.tile_pool(name="sbp", bufs=2))
            aggrp = es.enter_context(tc.tile_pool(name="aggrp", bufs=8))
            aggsp = es.enter_context(tc.tile_pool(name="aggsp", bufs=4))
            rp = es.enter_context(tc.tile_pool(name="rp", bufs=5))
            dp = es.enter_context(tc.tile_pool(name="dp", bufs=8))
            smalls = es.enter_context(tc.tile_pool(name="smalls", bufs=3))
            pk1 = es.enter_context(tc.tile_pool(name="pk1", bufs=1))
            pk2 = es.enter_context(tc.tile_pool(name="pk2", bufs=2))
            outp = es.enter_context(tc.tile_pool(name="outp", bufs=3))
            # ---- constants -------------------------------------------------
            ones2 = consts.tile([128, 2], _F16)
            nc.vector.memset(ones2, 1.0)
            # fp8 all-ones stationary for DoubleRow degree matmuls; sliced
            # [:, :, 0:2] (free strides must be 16B-aligned, hence width 16).
            ones2dr = consts.tile([128, 2, 16], _F8)
            nc.vector.memset(ones2dr, 1.0)
            blk = consts.tile([128, 2], _F16)      # block mean weights (1/64)
            nc.gpsimd.dma_start(out=blk, in_=blkc[:, :])
            sel = consts.tile([2, 128], _F16)      # mu/rstd partition-bcast
            nc.gpsimd.dma_start(out=sel, in_=selc[:, :])
            blkb = consts.tile([128, 128], _F16)   # kron(I2, J64)/64: r->mu_b
            nc.gpsimd.dma_start(out=blkb, in_=blkbc[:, :])
            sel4 = consts.tile([128, 128], _F16)   # sel replicated at 32k offs
            nc.gpsimd.dma_start(out=sel4, in_=sel4c[:, :])
            eps2 = consts.tile([2, 1], _F32)
            nc.gpsimd.dma_start(out=eps2, in_=epsc[:, :])

            # ---- DMA kickoff (ath on SP ring, xr on ACT ring, rest on
            # gpsimd ring).  at piece 0 is emitted first: it gates the first
            # matmul.
            at_lut = {h: [] for h in range(NH)}
            at_pieces = {h: [] for h in range(NH)}
            xr_lut = []
            aoff = 0
            xoff = 0
            for k, sz in enumerate(SLICES):
                for h in range(NH):
                    t = atp.tile([128, sz, 512], _F8, name=f"at{h}_{k}",
                                 tag="at", padded_shape=[128, 8, 512])
                    nc.sync.dma_start(out=t, in_=ath[h, :, aoff:aoff + sz, :])
                    at_lut[h].extend((t, l) for l in range(sz))
                    at_pieces[h].append((t, sz))
                aoff += sz
                t = xrp.tile([128, sz, BF], _F8, name=f"xr{k}", tag="xr",
                             padded_shape=[128, 8, BF])
                nc.scalar.dma_start(out=t, in_=xr[:, xoff:xoff + sz, :])
                xr_lut.extend((t, l) for l in range(sz))
                xoff += sz
            wn_sb = consts.tile([128, 128], _F16)
            nc.gpsimd.dma_start(out=wn_sb, in_=wnblk[:, :])
            ws_sb = consts.tile([128, 128], _F16)
            nc.gpsimd.dma_start(out=ws_sb, in_=wsblk[:, :])
            bias_sb = consts.tile([128, 1], _F32)
            nc.gpsimd.dma_start(out=bias_sb, in_=bvec[:, :])
            xt_sb = []
            for ch in range(CH):
                t = xtp.tile([128, R], _F16, name=f"xt{ch}", tag="xt")
                nc.gpsimd.dma_start(out=t, in_=xt2[ch])
                xt_sb.append(t)

            ft = [ftp.tile([128, 512], _F16, name=f"ft{h}", tag="ft")
                  for h in range(NH)]

            # ---- agg phase: 256 DoubleRow MMs + DVE degree folds -----------
            with tc.tile_pool(name="ps_agg", bufs=8, space="PSUM") as ps_agg:
                aggps = {(ch, h): ps_agg.tile([128, BF], _F32,
                                              name=f"agg{ch}{h}", tag="agg")
                         for ch in range(CH) for h in range(NH)}
                for q in range(JP):
                    xt_t, xl = xr_lut[2 * q]
                    xt_t2, xl2 = xr_lut[2 * q + 1]
                    assert xt_t2 is xt_t and xl2 == xl + 1
                    for ch in range(CH):
                        lhsT = xt_t[:, xl:xl + 2, ch * 128:(ch + 1) * 128]
                        for h in range(NH):
                            at_t, al = at_lut[h][2 * q]
                            at_t2, al2 = at_lut[h][2 * q + 1]
                            assert at_t2 is at_t and al2 == al + 1
                            nc.tensor.matmul(
                                aggps[(ch, h)], lhsT=lhsT,
                                rhs=at_t[:, al:al + 2, :],
                                start=(q == 0), stop=(q == JP - 1),
                                perf_mode=_DR,
                            )
                # Degree partial fold on the DVE for DMA pieces 0..4 (24 jt
                # per half), as a batched pair-sum tree (fp8 reads run at the
                # DVE's 1x tier, so batch FD large and keep levels shallow;
                # fp16 intermediates, counts <= 24, exact).  Pieces 5..9 are
                # handled by PE ones-matmuls in the backend scope.
                for h in range(NH):
                    phs = []
                    for k in range(FOLD_PIECES):
                        t, sz = at_pieces[h][k]
                        hf = sz // 2
                        ph = php.tile([128, hf, 512], _F8, tag="ph",
                                      padded_shape=[128, 4, 512])
                        nc.vector.tensor_add(out=ph, in0=t[:, 0:hf, :],
                                             in1=t[:, hf:sz, :])
                        phs.append((ph, hf))
                    # sizes now [1, 1, 2, 4, 4]; combine into ft[h] [128,512].
                    q1 = qrp.tile([128, 4, 512], _F16, tag="q")
                    nc.vector.tensor_add(out=q1, in0=phs[3][0], in1=phs[4][0])
                    r1 = qrp.tile([128, 2, 512], _F16, tag="r")
                    nc.vector.tensor_add(out=r1, in0=q1[:, 0:2, :],
                                         in1=q1[:, 2:4, :])
                    nc.vector.tensor_add(out=r1, in0=r1, in1=phs[2][0])
                    nc.vector.tensor_add(out=ft[h], in0=phs[0][0][:, 0, :],
                                         in1=phs[1][0][:, 0, :])
                    nc.vector.tensor_add(out=ft[h], in0=ft[h],
                                         in1=r1[:, 0, :])
                    nc.vector.tensor_add(out=ft[h], in0=ft[h],
                                         in1=r1[:, 1, :])

                # drain aggT to SBUF fp16 (raw; 1/deg applied later) to free
                # the PSUM banks for the backend pools.
                aggR = {}
                for ch in range(CH):
                    for h in range(NH):
                        t = aggrp.tile([128, BF], _F16, name=f"aR{ch}{h}",
                                       tag="aggR")
                        nc.scalar.copy(out=t, in_=aggps[(ch, h)])
                        aggR[(ch, h)] = t


            # ---- deg -> s = 1/max(deg,1), then backend pieces --------------
            with ExitStack() as es2:
                ps_tot = es2.enter_context(tc.tile_pool(name="ps_tot", bufs=2, space="PSUM"))
                ps_sm = es2.enter_context(tc.tile_pool(name="ps_sm", bufs=6, space="PSUM"))
                # Degree -> s = 1/max(deg, 1).  The DVE's iterative
                # reciprocal costs ~3.3us per call regardless of how few
                # partitions carry data, so both halves' clamped degrees are
                # packed into one [128, 512] tile at partition offsets 0/32
                # and inverted with a single call.
                dpk = pk1.tile([128, 512], _F32, tag="dpk")
                nc.vector.memset(dpk, 1.0)
                for h in range(NH):
                    # deg = folded pieces (via ones2 @ ft) + DoubleRow
                    # ones-matmuls over the unfolded at pieces 5..9.
                    degp = ps_sm.tile([2, 512], _F32, tag="pss",
                                      padded_shape=[128, 512])
                    nc.tensor.matmul(degp, lhsT=ones2, rhs=ft[h],
                                     start=True, stop=False,
                                     skip_group_check=True)
                    for jp in range(FOLD_JT // 2, JP):
                        at_t, al = at_lut[h][2 * jp]
                        at_t2, al2 = at_lut[h][2 * jp + 1]
                        assert at_t2 is at_t and al2 == al + 1
                        nc.tensor.matmul(
                            degp, lhsT=ones2dr[:, :, 0:2],
                            rhs=at_t[:, al:al + 2, :],
                            start=False, stop=(jp == JP - 1),
                            perf_mode=_DR, skip_group_check=True)
                    nc.vector.tensor_scalar_max(
                        out=dpk[32 * h:32 * h + 2, :], in0=degp, scalar1=1.0)
                spk = pk1.tile([128, 512], _F32, tag="spk")
                nc.vector.reciprocal(out=spk, in_=dpk)
                s_all = pk1.tile([128, 512], _F16, tag="s_all")
                nc.scalar.copy(out=s_all, in_=spk)
                s_b = []
                for h in range(NH):
                    sbb = ps_sm.tile([128, 512], _F32, name=f"s_bp{h}",
                                     tag="pss")
                    nc.tensor.matmul(sbb, lhsT=sel4[32 * h:32 * h + 2, :],
                                     rhs=s_all[32 * h:32 * h + 2, :],
                                     start=True, stop=True,
                                     tile_position=(32 * h, 0))
                    sbs = sbp.tile([128, 512], _F16, name=f"s_b{h}",
                                   tag="s_b")
                    nc.scalar.copy(out=sbs, in_=sbb)
                    s_b.append(sbs)

                # Backend pieces, emitted stage-major (engine queues are
                # strict FIFO; depth-first emission head-of-line-blocks every
                # later piece).  mu is broadcast per node in ONE matmul with
                # the kron(I2, J64)/64 stationary; rstd reciprocals are
                # batched 4 pieces per call at partition offsets 0/32/64/96.
                pieces = [(ch, h) for ch in range(CH) for h in range(NH)]
                st = {}
                for grp in range(2):
                    gp = list(range(grp * 4, grp * 4 + 4))
                    vpk = pk2.tile([128, 512], _F32, tag="vpk")
                    nc.vector.memset(vpk, 1.0)
                    for k in gp:
                        ch, h = pieces[k]
                        aggS = aggsp.tile([128, BF], _F16, tag="aggS")
                        nc.vector.tensor_mul(out=aggS, in0=aggR[(ch, h)],
                                             in1=s_b[h])
                        st[k] = {"aggS": aggS}
                    for k in gp:
                        ch, h = pieces[k]
                        tot = ps_tot.tile([128, 512], _F32, tag="tot")
                        nc.tensor.matmul(tot, lhsT=wn_sb, rhs=st[k]["aggS"],
                                         start=True, stop=False)
                        nc.tensor.matmul(
                            tot, lhsT=ws_sb,
                            rhs=xt_sb[ch][:, h * 512:(h + 1) * 512],
                            start=False, stop=True)
                        st[k]["tot"] = tot
                    for k in gp:
                        r = rp.tile([128, 512], _F16, tag="r")
                        nc.scalar.activation(
                            out=r, in_=st[k]["tot"],
                            func=mybir.ActivationFunctionType.Relu,
                            bias=bias_sb)
                        st[k]["r"] = r
                    for k in gp:
                        mu_b = ps_sm.tile([128, 512], _F32, tag="pss")
                        nc.tensor.matmul(mu_b, lhsT=blkb, rhs=st[k]["r"],
                                         start=True, stop=True)
                        st[k]["mu_b"] = mu_b
                    for k in gp:
                        d = dp.tile([128, 512], _F16, tag="d")
                        nc.vector.tensor_sub(out=d, in0=st[k]["r"],
                                             in1=st[k]["mu_b"])
                        st[k]["d"] = d
                    for k in gp:
                        d2 = rp.tile([128, 512], _F16, tag="d2")
                        nc.scalar.activation(
                            out=d2, in_=st[k]["d"],
                            func=mybir.ActivationFunctionType.Square)
                        st[k]["d2"] = d2
                    for k in gp:
                        var = ps_sm.tile([2, 512], _F32, tag="pss",
                                         padded_shape=[128, 512])
                        nc.tensor.matmul(var, lhsT=blk, rhs=st[k]["d2"],
                                         start=True, stop=True)
                        kk = k % 4
                        nc.vector.tensor_scalar_add(
                            out=vpk[32 * kk:32 * kk + 2, :], in0=var,
                            scalar1=LN_EPS)
                    sdp = pk2.tile([128, 512], _F32, tag="sdp")
                    nc.scalar.activation(
                        out=sdp, in_=vpk,
                        func=mybir.ActivationFunctionType.Sqrt)
                    rpk = pk2.tile([128, 512], _F32, tag="rpk")
                    nc.vector.reciprocal(out=rpk, in_=sdp)
                    rsa = pk2.tile([128, 512], _F16, tag="rsa")
                    nc.scalar.copy(out=rsa, in_=rpk)
                    st[grp * 4]["rsa"] = rsa
                for grp in range(2):
                    gp = list(range(grp * 4, grp * 4 + 4))
                    rsa = st[grp * 4]["rsa"]
                    for k in gp:
                        kk = k % 4
                        rstd_b = ps_sm.tile([128, 512], _F32, tag="pss")
                        nc.tensor.matmul(
                            rstd_b, lhsT=sel4[32 * kk:32 * kk + 2, :],
                            rhs=rsa[32 * kk:32 * kk + 2, :],
                            start=True, stop=True,
                            tile_position=(32 * kk, 0))
                        st[k]["rstd_b"] = rstd_b
                    for k in gp:
                        ch, h = pieces[k]
                        osb = outp.tile([128, 512], _F32, tag="osb")
                        nc.vector.tensor_mul(out=osb, in0=st[k]["d"],
                                             in1=st[k]["rstd_b"])
                        nc.sync.dma_start(
                            out=outT[ch][:, h * 512:(h + 1) * 512],
                            in_=osb)

    return nc


def _split_multi_waits(nc: bass.Bass) -> None:
    """This walrus build rejects any instruction carrying more than one sync
    wait ("Too many sync wait commands").  Tile's wait emission is per-proc
    minimal but not transitively so, and happily puts several waits on one
    instruction.  Equivalent fix: peel all but the last wait onto same-engine
    NOPs issued immediately before it (engine queues are strict FIFO, so the
    sequencer blocks on each in turn)."""
    from concourse.mybir import SyncInfo

    nid = 0
    for blk in nc.m.functions[0].blocks:
        out = []
        for inst in blk.instructions:
            si = getattr(inst, "sync_info", None)
            if si is not None and len(si.on_wait) > 1:
                waits = list(si.on_wait)
                for w in waits[:-1]:
                    nop = mybir.InstNoOp(name=f"wait_nop_{nid}")
                    nid += 1
                    nop.engine = inst.engine
                    nop.sync_info = SyncInfo(on_wait=[w], on_update=[])
                    out.append(nop)
                inst.sync_info = SyncInfo(
                    on_wait=[waits[-1]],
                    on_update=list(si.on_update),
                )
            out.append(inst)
        blk.instructions[:] = out


_NC_CACHE = None


def _get_nc() -> bass.Bass:
    global _NC_CACHE
    if _NC_CACHE is None:
        _NC_CACHE = _build_bass()
        _split_multi_waits(_NC_CACHE)
    return _NC_CACHE


def _prep_inputs(x, adj_matrix, W_self, W_neigh, b_self, b_neigh):
    """Host-side shard + layout prep (no reference math, just layout/dtype)."""
    x = np.asarray(x, dtype=np.float32)
    adj = np.asarray(adj_matrix)

    # xr[p, jt, b*64+f] = x[b, jt*128+p, f]; replicated to all cores.
    xr2 = x.transpose(1, 0, 2).reshape(N, BF)          # [j, bf]
    xr_host = np.ascontiguousarray(
        xr2.reshape(JT, 128, BF).transpose(1, 0, 2)
    ).astype(ml_dtypes.float8_e4m3fn)                  # [128 p, JT, BF]

    # kron(I2, W): block-diag weight for the 2-batches-per-chunk layout.
    wn_blk = np.kron(np.eye(2, dtype=np.float32), np.asarray(W_neigh, np.float32))
    ws_blk = np.kron(np.eye(2, dtype=np.float32), np.asarray(W_self, np.float32))
    wn_blk = np.ascontiguousarray(wn_blk).astype(np.float16)
    ws_blk = np.ascontiguousarray(ws_blk).astype(np.float16)

    # Pre-relu bias, per (b_local, f') partition: b_self + b_neigh.
    bv = (np.asarray(b_self, np.float32) + np.asarray(b_neigh, np.float32))
    bvec = np.tile(bv, 2).reshape(128, 1).astype(np.float32)

    # LN helpers: block-diag mean weights and the partition-bcast selector.
    blk_c = np.kron(np.eye(2, dtype=np.float32), np.ones((64, 1), np.float32))
    blk_c = (blk_c / 64.0).astype(np.float16)           # [128, 2]
    sel_c = np.kron(np.eye(2, dtype=np.float32),
                    np.ones((1, 64), np.float32)).astype(np.float16)  # [2, 128]
    eps_c = np.full((2, 1), LN_EPS, np.float32)
    # mu broadcast stationary: mu_b = blkb^T @ r with blkb = kron(I2, J64)/64.
    blkb_c = (np.kron(np.eye(2, dtype=np.float32),
                      np.ones((64, 64), np.float32)) / 64.0
              ).astype(np.float16)                      # [128, 128]
    # sel replicated at partition offsets 0/32/64/96 for batched bcasts.
    sel4_c = np.zeros((128, 128), np.float32)
    for kq in range(4):
        sel4_c[32 * kq:32 * kq + 2, :] = sel_c
    sel4_c = sel4_c.astype(np.float16)

    in_maps = []
    for c in range(N_CORES):
        rows = slice(c * R, (c + 1) * R)
        # ath[h, p, jt, i] = A[c*1024 + h*512 + i, jt*128 + p]
        a_c = adj[rows].reshape(NH, 512, JT, 128)       # [h, i, jt, p]
        ath_c = np.ascontiguousarray(
            a_c.transpose(0, 3, 2, 1)
        ).astype(ml_dtypes.float8_e4m3fn)               # [h, p, jt, i]

        # xt2[ch, p, i] = xr2[c*1024 + i, ch*128 + p]
        xb = xr2[rows].reshape(R, CH, 128)              # [i, ch, p]
        xt2_c = np.ascontiguousarray(
            xb.transpose(1, 2, 0)
        ).astype(np.float16)                            # [ch, p, i]

        in_maps.append({
            "xr": xr_host,
            "ath": ath_c,
            "xt2": xt2_c,
            "wnblk": wn_blk,
            "wsblk": ws_blk,
            "bvec": bvec,
            "blkc": blk_c,
            "selc": sel_c,
            "epsc": eps_c,
            "blkbc": blkb_c,
            "sel4c": sel4_c,
        })
    return in_maps


def _run(inputs: dict, trace: bool = False):
    x = np.asarray(inputs["x"], dtype=np.float32)
    in_maps = _prep_inputs(
        x, inputs["adj_matrix"], inputs["W_self"], inputs["W_neigh"],
        inputs["b_self"], inputs["b_neigh"],
    )
    nc = _get_nc()
    res = run_bass_kernel_spmd(nc, in_maps, core_ids=list(range(N_CORES)), trace=trace)

    out_full = np.empty((B, N, F), dtype=np.float32)
    for c in range(N_CORES):
        oc = res.results[c]["outT"]                     # [CH, 128, R] fp32
        out_full[:, c * R:(c + 1) * R, :] = (
            oc.reshape(BF, R).reshape(B, F, R).transpose(0, 2, 1)
        )

    # Exact host-side affine epilogue (gamma/beta are data, not compile-time).
    gamma = np.asarray(inputs["ln_gamma"], np.float32)
    beta = np.asarray(inputs["ln_beta"], np.float32)
    if not (np.all(gamma == 1.0) and np.all(beta == 0.0)):
        out_full = out_full * gamma + beta
    return out_full, res


def kernel(**inputs) -> np.ndarray:
    out, _ = _run(inputs, trace=False)
    return out


# revision 46
# speedup vs baseline: 1.1996x; 1.0045x over previous
"""GCNBlock (GraphSAGE mean conv + LayerNorm) Trainium2 kernel, v2.

Problem shapes (hardcoded): B=8, N=8192, F_IN=F_OUT=64, 8 NeuronCores.

Math (reference):
    A    = (adj > 0)                      # [N, N], values in {0, 1}
    deg  = A.sum(1)
    agg  = (A @ x[b]) / max(deg, 1)       # per batch b
    out  = relu(x @ W_self + agg @ W_neigh (+ biases))
    out  = LayerNorm(out) * gamma + beta  # over feature dim, eps=1e-5

Sharding: 1D row partition; core c owns node rows [c*1024, (c+1)*1024).

v2 design (vs v1's 163 us):
  * The big A@x aggregation runs in fp8 DoubleRow perf mode (2 fp8 MACs per
    PE cell per cycle): both the adjacency and x are fp8e4m3.  x-quantization
    error is benign because the neigh path is ~64x smaller in magnitude than
    the self path (its W output scale is 1/sqrt(deg) vs 1).  Measured DR MM
    cadence: 216 ns for a [128x(2x128)] x [128x(2x512)] matmul = 1 virtual
    column/cycle = fp8 peak; the 55 us agg phase is the compute floor.
  * Operands are SWAPPED vs v1: xr (x in [j, bf] layout) is the stationary,
    the adjacency streams as the moving operand.  The product then lands
    already transposed (aggT[bf, i]) which kills all 64 PE transposes of v1,
    and each stationary serves 2 matmuls so LDWEIGHTS (256-col DR load, no
    FWL) hides completely under the MM stream.
  * deg: at-tiles are pair-summed on the (otherwise idle) DVE into ft[j,i]
    per 512-node half, then one ones-matmul reduces the partition dim.
  * LayerNorm runs in the transposed domain: mean and centered variance are
    feature-dim reductions = tiny [128,2] block-diagonal matmuls on the PE;
    per-node (free-dim) broadcasts of mu/rstd are [2,128] selector matmuls
    into PSUM, not DVE work.  Per-piece DVE is only 3 elementwise ops.
  * Output is written transposed ([bf, i]) and unshuffled on the host.

Schedule: agg phase (256 MMs back-to-back, folds riding on DVE, at/xr
streaming on separate rings) -> deg reduce + 1/max(deg,1) -> 8 backend
pieces (chunk x node-half) pipelined across PE/ACT/DVE.

gamma/beta are applied on the host (exact affine; ones/zeros here).

HW exec time: see test.py; target ~80 us (PE ~66 us busy).
"""

import numpy as np
import ml_dtypes

import concourse.bass as bass
import concourse.mybir as mybir
from concourse.tile import TileContext
from concourse.bass_utils import run_bass_kernel_spmd

B, N, F = 8, 8192, 64
N_CORES = 8
R = N // N_CORES          # rows (nodes) per core = 1024
JT = N // 128             # contraction tiles = 64
JP = JT // 2              # DoubleRow contraction pairs = 32
BF = B * F                # stacked batch*feature dim = 512
CH = BF // 128            # 128-wide chunks of the bf dim = 4
NH = 2                    # 512-node halves of the core's rows
LN_EPS = 1e-5

_F16 = mybir.dt.float16
_F32 = mybir.dt.float32
_F8 = mybir.dt.float8e4
_DR = mybir.MatmulPerfMode.DoubleRow


def _build_bass() -> bass.Bass:
    nc = bass.Bass()

    # Host-side layouts (see _prep_inputs):
    #   xr : [128 p, JT, BF]      fp8, xr[p, jt, b*64+f] = x[b, jt*128+p, f]
    #   ath: [NH, 128 p, JT, 512] fp8, ath[h, p, jt, i] = A[c*1024+h*512+i, jt*128+p]
    #   xt2: [CH, 128 p, R]       fp16, xt2[ch, p, i] = x^T in chunk layout
    xr = nc.dram_tensor("xr", (128, JT, BF), _F8, kind="ExternalInput")
    ath = nc.dram_tensor("ath", (NH, 128, JT, 512), _F8, kind="ExternalInput")
    xt2 = nc.dram_tensor("xt2", (CH, 128, R), _F16, kind="ExternalInput")
    wnblk = nc.dram_tensor("wnblk", (128, 128), _F16, kind="ExternalInput")
    wsblk = nc.dram_tensor("wsblk", (128, 128), _F16, kind="ExternalInput")
    bvec = nc.dram_tensor("bvec", (128, 1), _F32, kind="ExternalInput")
    blkc = nc.dram_tensor("blkc", (128, 2), _F16, kind="ExternalInput")
    selc = nc.dram_tensor("selc", (2, 128), _F16, kind="ExternalInput")
    blkbc = nc.dram_tensor("blkbc", (128, 128), _F16, kind="ExternalInput")
    sel4c = nc.dram_tensor("sel4c", (128, 128), _F16, kind="ExternalInput")
    epsc = nc.dram_tensor("epsc", (2, 1), _F32, kind="ExternalInput")
    outT = nc.dram_tensor("outT", (CH, 128, R), _F32, kind="ExternalOutput")

    SLICES_AT = [2, 2, 4] + [4] * 14           # jt per at DMA piece (even)
    SLICES_XR = [2, 2, 4] + [4] * 14           # jt per xr DMA piece (even)
    FOLD_PIECES = 7                            # at pieces folded on the DVE
    FOLD_JT = sum(SLICES_AT[:FOLD_PIECES])     # = 24 jt (12 jp)

    from contextlib import ExitStack

    with TileContext(nc) as tc:
        with ExitStack() as es:
            consts = es.enter_context(tc.tile_pool(name="consts", bufs=1))
            xrp = es.enter_context(tc.tile_pool(name="xrp", bufs=len(SLICES_XR)))
            atp = es.enter_context(tc.tile_pool(name="atp", bufs=2 * len(SLICES_AT)))
            xtp = es.enter_context(tc.tile_pool(name="xtp", bufs=CH))
            ftp = es.enter_context(tc.tile_pool(name="ftp", bufs=NH))
            php = es.enter_context(tc.tile_pool(name="php", bufs=8))
            qrp = es.enter_context(tc.tile_pool(name="qrp", bufs=4))
            sbp = es.enter_context(tc.tile_pool(name="sbp", bufs=2))
            aggrp = es.enter_context(tc.tile_pool(name="aggrp", bufs=8))
            aggsp = es.enter_context(tc.tile_pool(name="aggsp", bufs=4))
            rp = es.enter_context(tc.tile_pool(name="rp", bufs=5))
            dp = es.enter_context(tc.tile_pool(name="dp", bufs=8))
            smalls = es.enter_context(tc.tile_pool(name="smalls", bufs=3))
            pk1 = es.enter_context(tc.tile_pool(name="pk1", bufs=1))
            pk2 = es.enter_context(tc.tile_pool(name="pk2", bufs=2))
            outp = es.enter_context(tc.tile_pool(name="outp", bufs=3))
            # ---- constants -------------------------------------------------
            ones2 = consts.tile([128, 2], _F16)
            nc.vector.memset(ones2, 1.0)
            # fp8 all-ones stationary for DoubleRow degree matmuls; sliced
            # [:, :, 0:2] (free strides must be 16B-aligned, hence width 16).
            ones2dr = consts.tile([128, 2, 16], _F8)
            nc.vector.memset(ones2dr, 1.0)
            blk = consts.tile([128, 2], _F16)      # block mean weights (1/64)
            nc.gpsimd.dma_start(out=blk, in_=blkc[:, :])
            sel = consts.tile([2, 128], _F16)      # mu/rstd partition-bcast
            nc.gpsimd.dma_start(out=sel, in_=selc[:, :])
            blkb = consts.tile([128, 128], _F16)   # kron(I2, J64)/64: r->mu_b
            nc.gpsimd.dma_start(out=blkb, in_=blkbc[:, :])
            sel4 = consts.tile([128, 128], _F16)   # sel replicated at 32k offs
            nc.gpsimd.dma_start(out=sel4, in_=sel4c[:, :])
            eps2 = consts.tile([2, 1], _F32)
            nc.gpsimd.dma_start(out=eps2, in_=epsc[:, :])

            # ---- DMA kickoff (ath on SP ring, xr on ACT ring, rest on
            # gpsimd ring).  at piece 0 is emitted first: it gates the first
            # matmul.
            at_lut = {h: [] for h in range(NH)}
            at_pieces = {h: [] for h in range(NH)}
            xr_lut = []
            aoff = 0
            xoff = 0
            for k in range(max(len(SLICES_AT), len(SLICES_XR))):
                if k < len(SLICES_AT):
                    sz = SLICES_AT[k]
                    for h in range(NH):
                        t = atp.tile([128, sz, 512], _F8, name=f"at{h}_{k}",
                                     tag="at", padded_shape=[128, 4, 512])
                        nc.sync.dma_start(out=t,
                                          in_=ath[h, :, aoff:aoff + sz, :])
                        at_lut[h].extend((t, l) for l in range(sz))
                        at_pieces[h].append((t, sz))
                    aoff += sz
                if k < len(SLICES_XR):
                    sz = SLICES_XR[k]
                    t = xrp.tile([128, sz, BF], _F8, name=f"xr{k}", tag="xr",
                                 padded_shape=[128, 4, BF])
                    nc.scalar.dma_start(out=t, in_=xr[:, xoff:xoff + sz, :])
                    xr_lut.extend((t, l) for l in range(sz))
                    xoff += sz
            wn_sb = consts.tile([128, 128], _F16)
            nc.gpsimd.dma_start(out=wn_sb, in_=wnblk[:, :])
            ws_sb = consts.tile([128, 128], _F16)
            nc.gpsimd.dma_start(out=ws_sb, in_=wsblk[:, :])
            bias_sb = consts.tile([128, 1], _F32)
            nc.gpsimd.dma_start(out=bias_sb, in_=bvec[:, :])
            xt_sb = []
            for ch in range(CH):
                t = xtp.tile([128, R], _F16, name=f"xt{ch}", tag="xt")
                nc.gpsimd.dma_start(out=t, in_=xt2[ch])
                xt_sb.append(t)

            ft = [ftp.tile([128, 512], _F16, name=f"ft{h}", tag="ft")
                  for h in range(NH)]

            # ---- agg phase: 256 DoubleRow MMs + DVE degree folds -----------
            with tc.tile_pool(name="ps_agg", bufs=8, space="PSUM") as ps_agg:
                aggps = {(ch, h): ps_agg.tile([128, BF], _F32,
                                              name=f"agg{ch}{h}", tag="agg")
                         for ch in range(CH) for h in range(NH)}
                for q in range(JP):
                    xt_t, xl = xr_lut[2 * q]
                    xt_t2, xl2 = xr_lut[2 * q + 1]
                    assert xt_t2 is xt_t and xl2 == xl + 1
                    for ch in range(CH):
                        lhsT = xt_t[:, xl:xl + 2, ch * 128:(ch + 1) * 128]
                        for h in range(NH):
                            at_t, al = at_lut[h][2 * q]
                            at_t2, al2 = at_lut[h][2 * q + 1]
                            assert at_t2 is at_t and al2 == al + 1
                            nc.tensor.matmul(
                                aggps[(ch, h)], lhsT=lhsT,
                                rhs=at_t[:, al:al + 2, :],
                                start=(q == 0), stop=(q == JP - 1),
                                perf_mode=_DR,
                            )
                # Degree partial fold on the DVE for DMA pieces 0..6 (24 jt
                # per half) as a batched pair-sum tree (fp8 reads run at the
                # DVE's 1x tier, so batch FD large; fp16 intermediates,
                # counts <= 24, exact).  Pieces 7..16 are handled by PE
                # ones-matmuls in the backend scope.
                for h in range(NH):
                    phs = []
                    for k in range(FOLD_PIECES):
                        t, sz = at_pieces[h][k]
                        hf = sz // 2
                        ph = php.tile([128, hf, 512], _F16, tag="ph",
                                      padded_shape=[128, 2, 512])
                        nc.vector.tensor_add(out=ph, in0=t[:, 0:hf, :],
                                             in1=t[:, hf:sz, :])
                        phs.append(ph)
                    # widths [1, 1, 2, 2, 2, 2, 2] -> ft [128, 512]
                    q1 = qrp.tile([128, 2, 512], _F16, tag="q")
                    nc.vector.tensor_add(out=q1, in0=phs[2], in1=phs[3])
                    q2 = qrp.tile([128, 2, 512], _F16, tag="q")
                    nc.vector.tensor_add(out=q2, in0=phs[4], in1=phs[5])
                    nc.vector.tensor_add(out=q1, in0=q1, in1=q2)
                    nc.vector.tensor_add(out=q1, in0=q1, in1=phs[6])
                    nc.vector.tensor_add(out=ft[h], in0=phs[0][:, 0, :],
                                         in1=phs[1][:, 0, :])
                    nc.vector.tensor_add(out=ft[h], in0=ft[h],
                                         in1=q1[:, 0, :])
                    nc.vector.tensor_add(out=ft[h], in0=ft[h],
                                         in1=q1[:, 1, :])

                # drain aggT to SBUF fp16 (raw; 1/deg applied later) to free
                # the PSUM banks for the backend pools.
                aggR = {}
                for ch in range(CH):
                    for h in range(NH):
                        t = aggrp.tile([128, BF], _F16, name=f"aR{ch}{h}",
                                       tag="aggR")
                        nc.scalar.copy(out=t, in_=aggps[(ch, h)])
                        aggR[(ch, h)] = t


            # ---- deg -> s = 1/max(deg,1), then backend pieces --------------
            with ExitStack() as es2:
                ps_tot = es2.enter_context(tc.tile_pool(name="ps_tot", bufs=2, space="PSUM"))
                ps_sm = es2.enter_context(tc.tile_pool(name="ps_sm", bufs=6, space="PSUM"))
                # Degree -> s = 1/max(deg, 1).  The DVE's iterative
                # reciprocal costs ~3.3us per call regardless of how few
                # partitions carry data, so both halves' clamped degrees are
                # packed into one [128, 512] tile at partition offsets 0/32
                # and inverted with a single call.
                dpk = pk1.tile([128, 512], _F32, tag="dpk")
                nc.vector.memset(dpk, 1.0)
                for h in range(NH):
                    # deg = folded pieces (via ones2 @ ft) + DoubleRow
                    # ones-matmuls over the unfolded at pieces 5..9.
                    degp = ps_sm.tile([2, 512], _F32, tag="pss",
                                      padded_shape=[128, 512])
                    nc.tensor.matmul(degp, lhsT=ones2, rhs=ft[h],
                                     start=True, stop=False,
                                     skip_group_check=True)
                    for jp in range(FOLD_JT // 2, JP):
                        at_t, al = at_lut[h][2 * jp]
                        at_t2, al2 = at_lut[h][2 * jp + 1]
                        assert at_t2 is at_t and al2 == al + 1
                        nc.tensor.matmul(
                            degp, lhsT=ones2dr[:, :, 0:2],
                            rhs=at_t[:, al:al + 2, :],
                            start=False, stop=(jp == JP - 1),
                            perf_mode=_DR, skip_group_check=True)
                    nc.vector.tensor_scalar_max(
                        out=dpk[32 * h:32 * h + 2, :], in0=degp, scalar1=1.0)
                spk = pk1.tile([128, 512], _F32, tag="spk")
                nc.vector.reciprocal(out=spk, in_=dpk)
                s_all = pk1.tile([128, 512], _F16, tag="s_all")
                nc.scalar.copy(out=s_all, in_=spk)
                s_b = []
                for h in range(NH):
                    sbb = ps_sm.tile([128, 512], _F32, name=f"s_bp{h}",
                                     tag="pss")
                    nc.tensor.matmul(sbb, lhsT=sel4[32 * h:32 * h + 2, :],
                                     rhs=s_all[32 * h:32 * h + 2, :],
                                     start=True, stop=True,
                                     tile_position=(32 * h, 0))
                    sbs = sbp.tile([128, 512], _F16, name=f"s_b{h}",
                                   tag="s_b")
                    nc.scalar.copy(out=sbs, in_=sbb)
                    s_b.append(sbs)

                # Backend pieces, emitted stage-major (engine queues are
                # strict FIFO; depth-first emission head-of-line-blocks every
                # later piece).  mu is broadcast per node in ONE matmul with
                # the kron(I2, J64)/64 stationary; rstd reciprocals are
                # batched 4 pieces per call at partition offsets 0/32/64/96.
                pieces = [(ch, h) for ch in range(CH) for h in range(NH)]
                st = {}
                for grp in range(2):
                    gp = list(range(grp * 4, grp * 4 + 4))
                    vpk = pk2.tile([128, 512], _F32, tag="vpk")
                    nc.vector.memset(vpk, 1.0)
                    for k in gp:
                        ch, h = pieces[k]
                        aggS = aggsp.tile([128, BF], _F16, tag="aggS")
                        nc.vector.tensor_mul(out=aggS, in0=aggR[(ch, h)],
                                             in1=s_b[h])
                        st[k] = {"aggS": aggS}
                    for k in gp:
                        ch, h = pieces[k]
                        tot = ps_tot.tile([128, 512], _F32, tag="tot")
                        nc.tensor.matmul(tot, lhsT=wn_sb, rhs=st[k]["aggS"],
                                         start=True, stop=False)
                        nc.tensor.matmul(
                            tot, lhsT=ws_sb,
                            rhs=xt_sb[ch][:, h * 512:(h + 1) * 512],
                            start=False, stop=True)
                        st[k]["tot"] = tot
                    for k in gp:
                        r = rp.tile([128, 512], _F16, tag="r")
                        nc.scalar.activation(
                            out=r, in_=st[k]["tot"],
                            func=mybir.ActivationFunctionType.Relu,
                            bias=bias_sb)
                        st[k]["r"] = r
                    for k in gp:
                        mu_b = ps_sm.tile([128, 512], _F32, tag="pss")
                        nc.tensor.matmul(mu_b, lhsT=blkb, rhs=st[k]["r"],
                                         start=True, stop=True)
                        st[k]["mu_b"] = mu_b
                    for k in gp:
                        d = dp.tile([128, 512], _F16, tag="d")
                        nc.vector.tensor_sub(out=d, in0=st[k]["r"],
                                             in1=st[k]["mu_b"])
                        st[k]["d"] = d
                    for k in gp:
                        d2 = rp.tile([128, 512], _F16, tag="d2")
                        nc.scalar.activation(
                            out=d2, in_=st[k]["d"],
                            func=mybir.ActivationFunctionType.Square)
                        st[k]["d2"] = d2
                    for k in gp:
                        var = ps_sm.tile([2, 512], _F32, tag="pss",
                                         padded_shape=[128, 512])
                        nc.tensor.matmul(var, lhsT=blk, rhs=st[k]["d2"],
                                         start=True, stop=True)
                        kk = k % 4
                        nc.vector.tensor_scalar_add(
                            out=vpk[32 * kk:32 * kk + 2, :], in0=var,
                            scalar1=LN_EPS)
                    sdp = pk2.tile([128, 512], _F32, tag="sdp")
                    nc.scalar.activation(
                        out=sdp, in_=vpk,
                        func=mybir.ActivationFunctionType.Sqrt)
                    rpk = pk2.tile([128, 512], _F32, tag="rpk")
                    nc.vector.reciprocal(out=rpk, in_=sdp)
                    rsa = pk2.tile([128, 512], _F16, tag="rsa")
                    nc.scalar.copy(out=rsa, in_=rpk)
                    st[grp * 4]["rsa"] = rsa
                for grp in range(2):
                    gp = list(range(grp * 4, grp * 4 + 4))
                    rsa = st[grp * 4]["rsa"]
                    for k in gp:
                        kk = k % 4
                        rstd_b = ps_sm.tile([128, 512], _F32, tag="pss")
                        nc.tensor.matmul(
                            rstd_b, lhsT=sel4[32 * kk:32 * kk + 2, :],
                            rhs=rsa[32 * kk:32 * kk + 2, :],
                            start=True, stop=True,
                            tile_position=(32 * kk, 0))
                        st[k]["rstd_b"] = rstd_b
                    for k in gp:
                        ch, h = pieces[k]
                        osb = outp.tile([128, 512], _F32, tag="osb")
                        nc.vector.tensor_mul(out=osb, in0=st[k]["d"],
                                             in1=st[k]["rstd_b"])
                        nc.sync.dma_start(
                            out=outT[ch][:, h * 512:(h + 1) * 512],
                            in_=osb)

    return nc


def _split_multi_waits(nc: bass.Bass) -> None:
    """This walrus build rejects any instruction carrying more than one sync
    wait ("Too many sync wait commands").  Tile's wait emission is per-proc
    minimal but not transitively so, and happily puts several waits on one
    instruction.  Equivalent fix: peel all but the last wait onto same-engine
    NOPs issued immediately before it (engine queues are strict FIFO, so the
    sequencer blocks on each in turn)."""
    from concourse.mybir import SyncInfo

    nid = 0
    for blk in nc.m.functions[0].blocks:
        out = []
        for inst in blk.instructions:
            si = getattr(inst, "sync_info", None)
            if si is not None and len(si.on_wait) > 1:
                waits = list(si.on_wait)
                for w in waits[:-1]:
                    nop = mybir.InstNoOp(name=f"wait_nop_{nid}")
                    nid += 1
                    nop.engine = inst.engine
                    nop.sync_info = SyncInfo(on_wait=[w], on_update=[])
                    out.append(nop)
                inst.sync_info = SyncInfo(
                    on_wait=[waits[-1]],
                    on_update=list(si.on_update),
                )
            out.append(inst)
        blk.instructions[:] = out


_NC_CACHE = None


def _get_nc() -> bass.Bass:
    global _NC_CACHE
    if _NC_CACHE is None:
        _NC_CACHE = _build_bass()
        _split_multi_waits(_NC_CACHE)
    return _NC_CACHE


def _prep_inputs(x, adj_matrix, W_self, W_neigh, b_self, b_neigh):
    """Host-side shard + layout prep (no reference math, just layout/dtype)."""
    x = np.asarray(x, dtype=np.float32)
    adj = np.asarray(adj_matrix)

    # xr[p, jt, b*64+f] = x[b, jt*128+p, f]; replicated to all cores.
    xr2 = x.transpose(1, 0, 2).reshape(N, BF)          # [j, bf]
    xr_host = np.ascontiguousarray(
        xr2.reshape(JT, 128, BF).transpose(1, 0, 2)
    ).astype(ml_dtypes.float8_e4m3fn)                  # [128 p, JT, BF]

    # kron(I2, W): block-diag weight for the 2-batches-per-chunk layout.
    wn_blk = np.kron(np.eye(2, dtype=np.float32), np.asarray(W_neigh, np.float32))
    ws_blk = np.kron(np.eye(2, dtype=np.float32), np.asarray(W_self, np.float32))
    wn_blk = np.ascontiguousarray(wn_blk).astype(np.float16)
    ws_blk = np.ascontiguousarray(ws_blk).astype(np.float16)

    # Pre-relu bias, per (b_local, f') partition: b_self + b_neigh.
    bv = (np.asarray(b_self, np.float32) + np.asarray(b_neigh, np.float32))
    bvec = np.tile(bv, 2).reshape(128, 1).astype(np.float32)

    # LN helpers: block-diag mean weights and the partition-bcast selector.
    blk_c = np.kron(np.eye(2, dtype=np.float32), np.ones((64, 1), np.float32))
    blk_c = (blk_c / 64.0).astype(np.float16)           # [128, 2]
    sel_c = np.kron(np.eye(2, dtype=np.float32),
                    np.ones((1, 64), np.float32)).astype(np.float16)  # [2, 128]
    eps_c = np.full((2, 1), LN_EPS, np.float32)
    # mu broadcast stationary: mu_b = blkb^T @ r with blkb = kron(I2, J64)/64.
    blkb_c = (np.kron(np.eye(2, dtype=np.float32),
                      np.ones((64, 64), np.float32)) / 64.0
              ).astype(np.float16)                      # [128, 128]
    # sel replicated at partition offsets 0/32/64/96 for batched bcasts.
    sel4_c = np.zeros((128, 128), np.float32)
    for kq in range(4):
        sel4_c[32 * kq:32 * kq + 2, :] = sel_c
    sel4_c = sel4_c.astype(np.float16)

    in_maps = []
    for c in range(N_CORES):
        rows = slice(c * R, (c + 1) * R)
        # ath[h, p, jt, i] = A[c*1024 + h*512 + i, jt*128 + p]
        a_c = adj[rows].reshape(NH, 512, JT, 128)       # [h, i, jt, p]
        ath_c = np.ascontiguousarray(
            a_c.transpose(0, 3, 2, 1)
        ).astype(ml_dtypes.float8_e4m3fn)               # [h, p, jt, i]

        # xt2[ch, p, i] = xr2[c*1024 + i, ch*128 + p]
        xb = xr2[rows].reshape(R, CH, 128)              # [i, ch, p]
        xt2_c = np.ascontiguousarray(
            xb.transpose(1, 2, 0)
        ).astype(np.float16)                            # [ch, p, i]

        in_maps.append({
            "xr": xr_host,
            "ath": ath_c,
            "xt2": xt2_c,
            "wnblk": wn_blk,
            "wsblk": ws_blk,
            "bvec": bvec,
            "blkc": blk_c,
            "selc": sel_c,
            "epsc": eps_c,
            "blkbc": blkb_c,
            "sel4c": sel4_c,
        })
    return in_maps


def _run(inputs: dict, trace: bool = False):
    x = np.asarray(inputs["x"], dtype=np.float32)
    in_maps = _prep_inputs(
        x, inputs["adj_matrix"], inputs["W_self"], inputs["W_neigh"],
        inputs["b_self"], inputs["b_neigh"],
    )
    nc = _get_nc()
    res = run_bass_kernel_spmd(nc, in_maps, core_ids=list(range(N_CORES)), trace=trace)

    out_full = np.empty((B, N, F), dtype=np.float32)
    for c in range(N_CORES):
        oc = res.results[c]["outT"]                     # [CH, 128, R] fp32
        out_full[:, c * R:(c + 1) * R, :] = (
            oc.reshape(BF, R).reshape(B, F, R).transpose(0, 2, 1)
        )

    # Exact host-side affine epilogue (gamma/beta are data, not compile-time).
    gamma = np.asarray(inputs["ln_gamma"], np.float32)
    beta = np.asarray(inputs["ln_beta"], np.float32)
    if not (np.all(gamma == 1.0) and np.all(beta == 0.0)):
        out_full = out_full * gamma + beta
    return out_full, res


def kernel(**inputs) -> np.ndarray:
    out, _ = _run(inputs, trace=False)
    return out
